# revision 47
# baseline (speedup 1.0000x reference)
"""Causal self-attention (B=2, T=2048, D=1024, H=16) on 8 NeuronCores.

Sharding: heads across cores (2 heads/core). Each core:
  - x arrives HOST-pretransposed ([D, B*T]) so x^T streams in as plain
    DMAs on the sync HWDGE ring (~330 GB/s, no DMA-transposes), t-ranges
    ordered by consumption; W_qkv is loaded per-qi between the first
    ranges so the first matmul starts ~10us in,
  - computes qT/kT/vT for its heads (W_qkv column shard), bf16 matmuls,
  - flash-style causal attention with scores transposed [k, q]; per-head
    score matmuls land on PE row-groups 0/64 and run concurrently,
  - diagonal blocks compute only the valid q-range (less exp + PE work);
    the triangular mask is one affine_select per diag block (both heads),
  - softmax sums via a ones-column on v; 1/sum via reciprocal_approx_fast
    (psum sums staged to SBUF first: the custom DVE op misreads PSUM),
  - b1 qkv, v-transposes and the projection are queued as fine-grained
    filler closures popped inside the attention kt-loop, ordered so no
    filler is queued before its x^T data can have arrived (premature
    fillers head-block the in-order PE/DVE queues),
  - partial projection (its 128 head-dims x full W_proj rows); psum
    drains split across Vector/Scalar; host sums the 8 partial outputs.
"""
import numpy as np
import ml_dtypes
from collections import deque
from contextlib import ExitStack

import concourse.bass as bass
import concourse.tile as tile
from concourse import bacc, mybir
from concourse.bass_utils import run_bass_kernel_spmd
from concourse.masks import make_identity

B, T, D, H, HD = 2, 2048, 1024, 16, 64
NCORES = 8
BT = B * T                    # 4096
DQ = 128                      # head dims per core (2 heads x 64)
TT = 512                      # t-tile for the qkv phase
NTT = BT // TT                # 8
NDC = D // 128                # 8 contraction chunks
NQS = T // 512                # 4 q-supertiles per batch
SCALE = 1.0 / np.sqrt(HD)     # 0.125
f32 = mybir.dt.float32
bf16 = mybir.dt.bfloat16
BF16NP = ml_dtypes.bfloat16


def build_module(debug=False):
    nc = bacc.Bacc("TRN2", target_bir_lowering=False, debug=False, num_devices=NCORES)
    # x arrives pre-transposed from host marshalling: [D, B*T]
    xt_d = nc.dram_tensor("xt", [D, BT], bf16, kind="ExternalInput").ap()
    wqkv_d = nc.dram_tensor("wqkv", [D, 3, DQ], bf16, kind="ExternalInput").ap()
    wp_d = nc.dram_tensor("wp", [128, D], bf16, kind="ExternalInput").ap()
    out_d = nc.dram_tensor("out", [BT, D], bf16, kind="ExternalOutput").ap()
    dbg = {}
    if debug:
        dbg = {
            "qT": nc.dram_tensor("dbg_qT", [128, BT], bf16, kind="ExternalOutput").ap(),
            "kT": nc.dram_tensor("dbg_kT", [128, BT], bf16, kind="ExternalOutput").ap(),
            "vext": nc.dram_tensor("dbg_vext", [128, BT // 128, 2, HD + 1], bf16, kind="ExternalOutput").ap(),
            "yTn": nc.dram_tensor("dbg_yTn", [128, BT], bf16, kind="ExternalOutput").ap(),
            "rec": nc.dram_tensor("dbg_rec", [16, 512], mybir.dt.float32, kind="ExternalOutput").ap(),
            "xt0": nc.dram_tensor("dbg_xt0", [128, BT], bf16, kind="ExternalOutput").ap(),
        }

    with tile.TileContext(nc) as tc, ExitStack() as ctx:
        const = ctx.enter_context(tc.tile_pool(name="const", bufs=1))
        wpool = ctx.enter_context(tc.tile_pool(name="w", bufs=1))
        big = ctx.enter_context(tc.tile_pool(name="big", bufs=1))
        vtp = ctx.enter_context(tc.tile_pool(name="vt", bufs=2))
        expp = ctx.enter_context(tc.tile_pool(name="exp", bufs=3))
        smal = ctx.enter_context(tc.tile_pool(name="small", bufs=4))
        outp = ctx.enter_context(tc.tile_pool(name="outsb", bufs=8))
        psum = ctx.enter_context(tc.tile_pool(name="ps", bufs=1, space="PSUM"))

        ident = const.tile([128, 128], bf16)
        make_identity(nc, ident[:])

        # Whole local W_qkv in SBUF: [d%128, d//128, {q,k,v}, dq].
        # On the sync ring BEFORE the x transposes: everything serializes
        # against DMA-transposes anyway, so order the critical loads first.
        wqkv_sb = wpool.tile([128, NDC, 3, DQ], bf16)

        def load_w(qi):
            nc.sync.dma_start(
                wqkv_sb[:, :, qi, :],
                wqkv_d[:, qi, :].rearrange("(c p) m -> p c m", p=128),
            )

        # This core's 128 rows of W_proj
        wp_sb = wpool.tile([128, D], bf16)
        nc.gpsimd.dma_start(wp_sb[:], wp_d[:])

        # x^T resident in SBUF: per d-chunk, [128 d, BT]. Plain DMAs of the
        # host-pretransposed x^T, t-ranges ordered by consumption; the k/v
        # weight loads ride between the first ranges.
        xtf = []
        for dc in range(NDC):
            xt = big.tile([128, BT], bf16, tag=f"xtf{dc}", name=f"xtf{dc}")
            xtf.append(xt)
        load_w(0)
        for r0, r1 in ((0, 512), (512, 1024), (1024, 2048), (2048, 3072), (3072, 4096)):
            for dc in range(NDC):
                nc.sync.dma_start(
                    xtf[dc][:, r0:r1],
                    xt_d[128 * dc : 128 * (dc + 1), r0:r1],
                )
            if r0 == 0:
                load_w(1)
                load_w(2)

        qT_sb = big.tile([128, BT], bf16)    # [2 heads x 64, b*T+t]
        kT_sb = big.tile([128, BT], bf16)
        # v natural + ones column: [k%128, k//128, head, hd+1]
        vext_sb = big.tile([128, BT // 128, 2, HD + 1], bf16)
        nc.gpsimd.memset(vext_sb[:, :, :, HD : HD + 1], 1.0)
        yTn_sb = big.tile([128, BT], bf16)   # normalized y^T

        # ---------------- filler units ----------------
        fillers = deque()

        def pop_fillers(n):
            for _ in range(n):
                if not fillers:
                    return
                fillers.popleft()()

        def qkv_half(tt, qi, half, ps_ref, on_scalar):
            # half 0: dc 0-3 into a fresh acc psum; half 1: dc 4-7 + cast out
            t0 = TT * tt
            if half == 0:
                ps = psum.tile([128, TT], f32, tag="acc", bufs=3, name=f"qk{tt}_{qi}")
                ps_ref.append(ps)
            ps = ps_ref[0]
            for dc in range(4 * half, 4 * half + 4):
                nc.tensor.matmul(
                    ps[:],
                    wqkv_sb[:, dc, qi, :],
                    xtf[dc][:, t0 : t0 + TT],
                    start=(dc == 0),
                    stop=(dc == NDC - 1),
                )
            if half == 1:
                if qi == 0:
                    dst = qT_sb[:, t0 : t0 + TT]
                elif qi == 1:
                    dst = kT_sb[:, t0 : t0 + TT]
                else:
                    dst = vtp.tile([128, TT], bf16, tag="vts", name=f"vts{tt}")
                    ps_ref.append(dst)  # keep for v_transposes
                if on_scalar:
                    nc.scalar.copy(dst, ps[:])
                else:
                    nc.vector.tensor_copy(dst, ps[:])

        def v_transposes(tt, vts, s_range):
            for s in s_range:
                vn_ps = psum.tile([128, 128], bf16, tag="vtp", bufs=1, name=f"vn{tt}{s}")
                nc.tensor.transpose(vn_ps[:], vts[:, 128 * s : 128 * (s + 1)], ident[:])
                kchunk = 4 * tt + s
                nc.vector.tensor_copy(
                    vext_sb[:, kchunk, :, 0:HD],
                    vn_ps[:].rearrange("p (h d) -> p h d", h=2),
                )

        def emit_qkv_tile(tt, on_scalar, via_filler):
            """Emit (or queue) the 7 filler units for one qkv t-tile."""
            refs = {qi: [] for qi in range(3)}
            units = []
            for qi in range(3):
                for half in range(2):
                    units.append(
                        lambda tt=tt, qi=qi, half=half, r=refs: qkv_half(
                            tt, qi, half, r[qi], on_scalar
                        )
                    )
            def vtr(tt=tt, r=refs):
                v_transposes(tt, r[2][1], range(4))
            units.append(vtr)
            if via_filler:
                fillers.extend(units)
            else:
                for u in units:
                    u()

        def proj_unit(j, ts, split_heads=False):
            # one 128-row chunk of output rows [512j+128ts, ...+128)
            c0 = 512 * j + 128 * ts
            osb = outp.tile([128, 2, 512], bf16, tag="osb", name=f"osb{j}_{ts}")
            for half in range(2):
                pp = psum.tile([128, 512], f32, tag="acc", bufs=3, name=f"pp{j}{ts}{half}")
                if split_heads:
                    # per-head K=64 stages: h0's matmul only needs h0's
                    # normalized yT, overlapping the other head's norm chain
                    for h in range(2):
                        nc.tensor.matmul(
                            pp[:],
                            yTn_sb[64 * h : 64 * h + 64, c0 : c0 + 128],
                            wp_sb[64 * h : 64 * h + 64, 512 * half : 512 * (half + 1)],
                            start=(h == 0),
                            stop=(h == 1),
                        )
                else:
                    nc.tensor.matmul(
                        pp[:],
                        yTn_sb[:, c0 : c0 + 128],
                        wp_sb[:, 512 * half : 512 * (half + 1)],
                        start=True,
                        stop=True,
                    )
                # ACT is ~50% idle during attention: split the psum drains
                if half == 0:
                    nc.vector.tensor_copy(osb[:, half, :], pp[:])
                else:
                    nc.scalar.copy(osb[:, half, :], pp[:])
            nc.sync.dma_start(out_d[c0 : c0 + 128, :], osb[:])

        def push_proj(j, front=False):
            if front:
                for ts in reversed(range(4)):
                    fillers.appendleft(lambda j=j, ts=ts: proj_unit(j, ts))
            else:
                for ts in range(4):
                    fillers.append(lambda j=j, ts=ts: proj_unit(j, ts))

        # ---------------- attention ----------------
        def emit_scores(b, qs, kt, qg):
            s = kt - 4 * qs  # >=0 on the diagonal supertile
            qoff = 128 * s if s > 0 else 0
            kg = 2048 * b + 128 * kt
            st_ps = psum.tile(
                [128, 2, 512], f32, tag="sc", bufs=2, name=f"st_{b}_{qs}_{kt}"
            )
            for h in range(2):
                nc.tensor.matmul(
                    st_ps[:, h, qoff:512],
                    kT_sb[64 * h : 64 * h + 64, kg : kg + 128],
                    qT_sb[64 * h : 64 * h + 64, qg + qoff : qg + 512],
                    start=True,
                    stop=True,
                )
            est = expp.tile([128, 2, 512], bf16, tag="est", name=f"est_{b}_{qs}_{kt}")
            nc.scalar.activation(
                est[:, :, qoff:512],
                st_ps[:, :, qoff:512],
                mybir.ActivationFunctionType.Exp,
                scale=SCALE,
            )
            if s >= 0:
                # triangular mask on the single partially-masked 128-q strip
                # (both heads in one op): keep where (q - k) = j - p >= 0
                nc.gpsimd.affine_select(
                    out=est[:, :, qoff : qoff + 128],
                    in_=est[:, :, qoff : qoff + 128],
                    pattern=[[0, 2], [1, 128]],
                    compare_op=mybir.AluOpType.is_ge,
                    fill=0.0,
                    base=0,
                    channel_multiplier=-1,
                )
            return est, qoff

        def attention_block(b, qs, pops_per_kt, pre=None, pops_from=0):
            qg = 2048 * b + 512 * qs
            nkt = 4 * qs + 4
            yt_ps0 = psum.tile([HD + 1, 512], f32, tag="acc", bufs=3, name=f"yt0_{b}_{qs}")
            yt_ps1 = psum.tile([HD + 1, 512], f32, tag="acc", bufs=3, name=f"yt1_{b}_{qs}")
            yt_ps = [yt_ps0, yt_ps1]
            est_next = emit_scores(b, qs, 0, qg)
            if pre is not None:
                pre()  # previous block's norm chain: front of the DVE queue
            for kt in range(nkt):
                est, qoff = est_next
                if kt + 1 < nkt:
                    est_next = emit_scores(b, qs, kt + 1, qg)
                kchunk = (2048 * b + 128 * kt) // 128
                for h in range(2):
                    nc.tensor.matmul(
                        yt_ps[h][:, qoff:512],
                        vext_sb[:, kchunk, h, :],
                        est[:, h, qoff:512],
                        start=(kt == 0),
                        stop=(kt == nkt - 1),
                    )
                if kt >= pops_from:
                    pop_fillers(pops_per_kt)

            def norm():
                # softmax normalization: sums live in row HD of each yt psum
                for h in range(2):
                    su = smal.tile([1, 512], f32, tag="su", name=f"su_{b}_{qs}_{h}")
                    nc.vector.tensor_copy(su[:], yt_ps[h][HD : HD + 1, :])
                    rec = smal.tile([1, 512], f32, tag="rec", name=f"rec_{b}_{qs}_{h}")
                    nc.vector.reciprocal_approx_fast(rec[:], su[:])
                    if debug:
                        nc.sync.dma_start(dbg["rec"][(4 * b + qs) * 2 + h, :], rec[:])
                    recb = smal.tile([1, 512], bf16, tag="recb", name=f"recb_{b}_{qs}_{h}")
                    nc.vector.tensor_copy(recb[:], rec[:])
                    bc = smal.tile([HD, 512], bf16, tag="bc", name=f"bc_{b}_{qs}_{h}")
                    nc.gpsimd.partition_broadcast(bc[:], recb[:])
                    nc.vector.tensor_mul(
                        yTn_sb[64 * h : 64 * h + 64, qg : qg + 512],
                        yt_ps[h][0:HD, :],
                        bc[:],
                    )

            return norm

        # ---------------- schedule ----------------
        # b0: qkv t0 direct; attention ascending (tracks x^T arrival);
        # qkv t1..t3 then b1's t4..t7 ride as fillers.
        emit_qkv_tile(0, on_scalar=True, via_filler=False)
        emit_qkv_tile(1, on_scalar=True, via_filler=True)
        nrm = attention_block(0, 0, pops_per_kt=2)
        pop_fillers(len(fillers))  # ensure t1 complete
        emit_qkv_tile(2, on_scalar=True, via_filler=True)
        emit_qkv_tile(3, on_scalar=True, via_filler=True)
        nrm = attention_block(0, 1, pops_per_kt=2, pre=nrm)
        pop_fillers(len(fillers))  # ensure t2/t3 complete
        # During (0,2), b1's x^T is still in flight — premature b1-qkv
        # filler MMs would head-block the PE/DVE queues. Fill with proj of
        # the finished blocks j0/j1 instead; b1 qkv drains during (0,3).
        push_proj(0)
        push_proj(1)
        nrm = attention_block(0, 2, pops_per_kt=1, pre=nrm)
        for tt in range(4, NTT):
            emit_qkv_tile(tt, on_scalar=False, via_filler=True)
        nrm = attention_block(0, 3, pops_per_kt=2, pre=nrm, pops_from=4)
        pop_fillers(len(fillers))  # ensure all b1 qkv complete
        # b0 y done: remaining b0 proj as fillers for b1 attention
        push_proj(2)
        push_proj(3)
        nrm = attention_block(1, 0, pops_per_kt=1, pre=nrm)
        push_proj(4)
        nrm = attention_block(1, 1, pops_per_kt=1, pre=nrm)
        push_proj(5)
        nrm = attention_block(1, 2, pops_per_kt=1, pre=nrm)
        push_proj(6)
        nrm = attention_block(1, 3, pops_per_kt=1, pre=nrm)
        nrm()  # final block's norm immediately
        push_proj(7)
        pop_fillers(len(fillers))

        if debug:
            nc.sync.dma_start(dbg["qT"][:], qT_sb[:])
            nc.sync.dma_start(dbg["kT"][:], kT_sb[:])
            nc.sync.dma_start(dbg["vext"][:], vext_sb[:])
            nc.sync.dma_start(dbg["yTn"][:], yTn_sb[:])
            nc.sync.dma_start(dbg["xt0"][:], xtf[0][:])

    nc.compile()
    return nc


_NC_CACHE = None


def _get_module():
    global _NC_CACHE
    if _NC_CACHE is None:
        _NC_CACHE = build_module()
    return _NC_CACHE


def make_in_maps(x, W_qkv, W_proj):
    xt = np.ascontiguousarray(
        np.asarray(x, dtype=np.float32).reshape(BT, D).astype(BF16NP).T
    )
    wq = np.asarray(W_qkv, dtype=np.float32)
    wp_full = np.asarray(W_proj, dtype=np.float32)
    in_maps = []
    for c in range(NCORES):
        wp = np.ascontiguousarray(wp_full[128 * c : 128 * (c + 1), :].astype(BF16NP))
        wl = np.ascontiguousarray(
            np.stack(
                [
                    wq[:, 128 * c : 128 * (c + 1)],
                    wq[:, D + 128 * c : D + 128 * (c + 1)],
                    wq[:, 2 * D + 128 * c : 2 * D + 128 * (c + 1)],
                ],
                axis=1,
            ).astype(BF16NP)
        )
        in_maps.append({"xt": xt, "wqkv": wl, "wp": wp})
    return in_maps


def run(x, W_qkv, W_proj, **spmd_kwargs):
    nc = _get_module()
    in_maps = make_in_maps(x, W_qkv, W_proj)
    res = run_bass_kernel_spmd(nc, in_maps, list(range(NCORES)), **spmd_kwargs)
    out = np.zeros((BT, D), dtype=np.float32)
    for c in range(NCORES):
        out += res.results[c]["out"].astype(np.float32)
    return out.reshape(B, T, D), res


def kernel(x, W_qkv, W_proj):
    out, _ = run(x, W_qkv, W_proj)
    return out
